# revision 20
# baseline (speedup 1.0000x reference)
"""ComplexMultiheadAttention on 8 Trainium2 NeuronCores.

Sharding: core c handles batch b = c//4 and the 4 heads [4*(c%4), 4*(c%4)+4).
QKV use Gauss's 3-multiplication complex trick: with A = wr[Ch].T and
B = wi[Ch].T, the three K=1024 chains  m2 = zr(B-A), m3 = zi(B+A), m1 = zsA
(zs = zr+zi precomputed on host) give real = m1-m3, imag = m1+m2 — 25% less
PE work than the fused K=2048 form.  The DVE folds bias + combine + PSUM
evacuation into one scalar_tensor_tensor op per 64-channel half.
The O-projection is row-parallel (Megatron): each core emits a partial
[2048,1024] sum; the host adds the 4 partials per batch plus the exact
bias term (V-bias folds into the output bias because softmax rows sum to 1).
"""

import os
import sys

import numpy as np

sys.path.insert(0, "/opt/trn_rl_repo")

import concourse.bass as bass
import concourse.bacc as bacc_mod
import concourse.mybir as mybir
from concourse.bass_utils import run_bass_kernel_spmd
from concourse.tile import TileContext

try:  # tracing needs antenv.axon_hooks (test harness injects it)
    import antenv.axon_hooks  # noqa: F401
except ImportError:
    os.environ.setdefault("BASS_NEVER_TRACE", "1")

B, L, D, NH = 2, 2048, 1024, 16
HD = D // NH  # 64
N_CORES = 8
NHL = 4  # heads per core
CH2 = 2 * NHL * HD  # 512 local channels, per-head [r(64), i(64)] interleaved
F2 = 2 * D  # 2048 concat feature dim
SCALE = 1.0 / 8.0  # 1/sqrt(HD)

F32 = mybir.dt.float32
F32R = mybir.dt.float32r
AF = mybir.ActivationFunctionType


def _build_nc():
    nc = bacc_mod.Bacc(None, target_bir_lowering=False, debug=False)
    # z3t rows: m=0: zr^T, m=1: zi^T, m=2: (zr+zi)^T
    z3t = nc.declare_dram_parameter("z3t", [3 * D, L], F32R, isOutput=False)
    # w?g rows (m*1024+f): m=0: (wi-wr)[Ch].T, m=1: (wi+wr)[Ch].T, m=2: wr[Ch].T
    wqg = nc.declare_dram_parameter("wqg", [3 * D, 256], F32R, isOutput=False)
    wkg = nc.declare_dram_parameter("wkg", [3 * D, 256], F32R, isOutput=False)
    wvg = nc.declare_dram_parameter("wvg", [3 * D, 256], F32R, isOutput=False)
    cqr = nc.declare_dram_parameter("cqr", [256], F32, isOutput=False)
    cqi = nc.declare_dram_parameter("cqi", [256], F32, isOutput=False)
    ckr = nc.declare_dram_parameter("ckr", [256], F32, isOutput=False)
    cki = nc.declare_dram_parameter("cki", [256], F32, isOutput=False)
    wor = nc.declare_dram_parameter("wor", [CH2, D], F32R, isOutput=False)
    woi = nc.declare_dram_parameter("woi", [CH2, D], F32R, isOutput=False)
    pr = nc.declare_dram_parameter("pr", [L, D], F32, isOutput=True)
    pi = nc.declare_dram_parameter("pi", [L, D], F32, isOutput=True)
    ALU = mybir.AluOpType

    NSB = 4
    SBW = L // NSB  # 512 seq cols per block

    with TileContext(nc) as tc:
        with (
            tc.tile_pool(name="dram", bufs=1, space="DRAM") as dpool,
            tc.tile_pool(name="persist", bufs=1) as pers,
        ):
            # per-head contiguous spill tiles -> single big-descriptor loads
            qt_parts = [
                dpool.tile([128, L], F32R, tag=f"qtp_{h}", name=f"qtp_{h}")
                for h in range(NHL)
            ]
            kt_parts = [
                dpool.tile([128, L], F32R, tag=f"ktp_{h}", name=f"ktp_{h}")
                for h in range(NHL)
            ]
            v_parts = [
                dpool.tile([128, 16, 128], F32R, tag=f"vp_{h}", name=f"vp_{h}")
                for h in range(1, NHL)
            ]

            ones_f = pers.tile([128, 1], F32, tag="ones_f")
            nc.vector.memset(ones_f[:], 1.0)
            ones = pers.tile([128, 1], F32R, tag="ones")
            nc.scalar.activation(ones[:], ones_f[:], AF.Copy)
            onesr_f = pers.tile([1, 128], F32, tag="onesr_f")
            nc.vector.memset(onesr_f[:], 1.0)
            onesr = pers.tile([1, 128], F32R, tag="onesr")
            nc.scalar.activation(onesr[:], onesr_f[:], AF.Copy)
            warm = pers.tile([128, 8], F32R, tag="warm")
            bias_sb = {}
            for nm, dram in (("cqr", cqr), ("cqi", cqi), ("ckr", ckr), ("cki", cki)):
                t = pers.tile([128, 2], F32, tag=nm)
                nc.sync.dma_start(t[:], dram[:].rearrange("(t p) -> p t", p=128))
                bias_sb[nm] = t

            # ---------- Phase 1: QKV projections ----------
            # QT/KT in [ch, seq] layout; V in [seq, ch] layout.
            # Head 0's Q/K land directly in SBUF (no DRAM round-trip) so
            # phase 2 starts with zero DMA latency.
            h0_ctx = tc.tile_pool(name="head0", bufs=1)
            h0pool = h0_ctx.__enter__()
            qt_h0 = h0pool.tile([128, L], F32R, tag="qt_h0")
            kt_h0 = h0pool.tile([128, L], F32R, tag="kt_h0")
            v_h0 = h0pool.tile([128, 16, 128], F32R, tag="v_h0")
            with (
                tc.tile_pool(name="w1", bufs=1) as wpool,
                tc.tile_pool(name="z3", bufs=10) as zpool,
                tc.tile_pool(name="ev1", bufs=4) as ev1,
                tc.tile_pool(name="evv", bufs=2) as evv,
                tc.tile_pool(name="ps_qk", bufs=4, space="PSUM") as ps_qk,
                tc.tile_pool(name="ps_v", bufs=4, space="PSUM") as ps_v,
            ):
                wq_sb = wpool.tile([128, 24, 256], F32R, tag="wq")
                wk_sb = wpool.tile([128, 24, 256], F32R, tag="wk")
                wv_sb = wpool.tile([128, 24, 256], F32R, tag="wv")

                zcs = {}  # (sb, m, half) -> [128, 4, SBW] chunk tile

                def z3_chunk(sb, m, half):
                    t = zpool.tile([128, 4, SBW], F32R, tag="zc")
                    nc.sync.dma_start(
                        t[:],
                        z3t[
                            m * D + half * 512 : m * D + (half + 1) * 512,
                            sb * SBW : (sb + 1) * SBW,
                        ].rearrange("(t p) s -> p t s", p=128),
                    )
                    zcs[(sb, m, half)] = t

                def wslice(wsb, wd, i):
                    nc.sync.dma_start(wsb[:, i, :], wd[i * 128 : (i + 1) * 128, :])

                # critical-prefix DMA order: first chain's chunk + its weight
                # slices go first so the PE starts after ~1.3 MB, with the
                # rest of the stream ordered by consumption.
                z3_chunk(0, 0, 0)
                for i in range(8):
                    wslice(wq_sb, wqg, i)
                z3_chunk(0, 0, 1)
                for i in range(8, 16):
                    wslice(wq_sb, wqg, i)
                z3_chunk(0, 1, 0)
                z3_chunk(0, 1, 1)
                for i in range(16, 24):
                    wslice(wq_sb, wqg, i)
                z3_chunk(0, 2, 0)
                z3_chunk(0, 2, 1)
                for i in range(24):
                    wslice(wk_sb, wkg, i)
                for i in range(24):
                    wslice(wv_sb, wvg, i)

                for sb in range(NSB):
                    if sb > 0:
                        # m0/m1 chunks were prefetched during sb-1's V section
                        for m, half in ((2, 0), (2, 1)):
                            z3_chunk(sb, m, half)
                    sc = slice(sb * SBW, (sb + 1) * SBW)
                    for wsb, cr, ci, parts, direct in (
                        (wq_sb, bias_sb["cqr"], bias_sb["cqi"], qt_parts, qt_h0),
                        (wk_sb, bias_sb["ckr"], bias_sb["cki"], kt_parts, kt_h0),
                    ):
                        for ct in range(2):
                            ps = [None] * 3
                            for m in range(3):
                                ps[m] = ps_qk.tile(
                                    [128, SBW], F32, tag="psqk", name=f"psqk_{m}"
                                )
                                for f8 in range(8):
                                    nc.tensor.matmul(
                                        ps[m][:],
                                        lhsT=wsb[
                                            :, m * 8 + f8, ct * 128 : (ct + 1) * 128
                                        ],
                                        rhs=zcs[(sb, m, f8 // 4)][:, f8 % 4, :],
                                        start=(f8 == 0),
                                        stop=(f8 == 7),
                                    )
                            # per head: real = m1-m3+cbr, imag = m1+m2+cbi
                            # (ps[2]=m1 zs-chain, ps[1]=m3 zi, ps[0]=m2 zr);
                            # m1 goes PSUM->SBUF first (TensorScalarPtr may
                            # read only one PSUM operand)
                            m1s = ev1.tile([128, SBW], F32R, tag="m1s", name="m1s")
                            nc.scalar.activation(m1s[:], ps[2][:], AF.Copy)
                            for hh in range(2):
                                h = 2 * ct + hh
                                pp = slice(hh * 64, (hh + 1) * 64)
                                if h == 0:
                                    dr = direct[0:64, sc]
                                    di = direct[64:128, sc]
                                else:
                                    stg = ev1.tile([128, SBW], F32R, tag="ev1")
                                    dr = stg[0:64, :]
                                    di = stg[64:128, :]
                                with nc.allow_low_precision(reason="f32r qkv"):
                                    nc.vector.scalar_tensor_tensor(
                                        dr,
                                        m1s[pp, :],
                                        cr[pp, ct : ct + 1],
                                        ps[1][pp, :],
                                        op0=ALU.add,
                                        op1=ALU.subtract,
                                    )
                                    nc.vector.scalar_tensor_tensor(
                                        di,
                                        m1s[pp, :],
                                        ci[pp, ct : ct + 1],
                                        ps[0][pp, :],
                                        op0=ALU.add,
                                        op1=ALU.add,
                                    )
                                if h > 0:
                                    nc.sync.dma_start(parts[h][:, sc], stg[:])
                    for st in range(SBW // 128):
                        if sb < NSB - 1 and st == 0:
                            # prefetch next block's first-chain chunks
                            z3_chunk(sb + 1, 0, 0)
                            z3_chunk(sb + 1, 0, 1)
                        if sb < NSB - 1 and st == 2:
                            z3_chunk(sb + 1, 1, 0)
                            z3_chunk(sb + 1, 1, 1)
                        psv = [None] * 3
                        for m in range(3):
                            psv[m] = ps_v.tile(
                                [128, 4, 64], F32, tag="psv", name=f"psv_{m}"
                            )
                            for f8 in range(8):
                                nc.tensor.matmul(
                                    psv[m][:],
                                    lhsT=zcs[(sb, m, f8 // 4)][
                                        :, f8 % 4, st * 128 : (st + 1) * 128
                                    ],
                                    rhs=wv_sb[:, m * 8 + f8, :],
                                    start=(f8 == 0),
                                    stop=(f8 == 7),
                                )
                        vstage = evv.tile([128, 4, 128], F32R, tag="vstage")
                        m1v = evv.tile([128, 4, 64], F32R, tag="m1v", name="m1v")
                        nc.scalar.activation(m1v[:], psv[2][:], AF.Copy)
                        with nc.allow_low_precision(reason="f32r v combine"):
                            nc.vector.scalar_tensor_tensor(
                                vstage[:, :, 0:64],
                                m1v[:],
                                0.0,
                                psv[1][:],
                                op0=ALU.add,
                                op1=ALU.subtract,
                            )
                            nc.vector.scalar_tensor_tensor(
                                vstage[:, :, 64:128],
                                m1v[:],
                                0.0,
                                psv[0][:],
                                op0=ALU.add,
                                op1=ALU.add,
                            )
                        ktidx = sb * 4 + st
                        nc.sync.dma_start(v_h0[:, ktidx, :], vstage[:, 0, :])
                        for hh in range(1, NHL):
                            nc.sync.dma_start(
                                v_parts[hh - 1][:, ktidx, :], vstage[:, hh, :]
                            )

            # ---------- Phase 2: attention, flat software pipeline ----------
            # Rowsums ride the (otherwise idle) DVE as a running elementwise
            # sum of the 16 exp strips; one ones-matmul per (h,qb) block
            # reduces the accumulated strip across partitions.
            # warm up the GpSimd custom-instruction library before phase 2
            # (first partition_broadcast otherwise pays a ~10us IRAM load)
            nc.gpsimd.partition_broadcast(warm[:], onesr[0:1, 0:8])
            wo_ctx = tc.tile_pool(name="wo", bufs=1)
            wopool = wo_ctx.__enter__()
            wor_sb = wopool.tile([128, NHL, D], F32R, tag="wor")
            woi_sb = wopool.tile([128, NHL, D], F32R, tag="woi")
            for hh in range(NHL):
                nc.sync.dma_start(wor_sb[:, hh, :], wor[hh * 128 : (hh + 1) * 128, :])
                nc.sync.dma_start(woi_sb[:, hh, :], woi[hh * 128 : (hh + 1) * 128, :])
            # per-qb OT tiles: [128 ch, head, 512 q], per-head [or(64), oi(64)]
            ot_qb = [
                wopool.tile([128, NHL, 512], F32R, tag=f"ot_{qb}", name=f"ot_{qb}")
                for qb in range(4)
            ]

            with (
                tc.tile_pool(name="heads", bufs=3) as hpool,
                tc.tile_pool(name="pstrips", bufs=6) as ppool,
                tc.tile_pool(name="accs", bufs=4) as apool,
                tc.tile_pool(name="small2", bufs=4) as spool,
                tc.tile_pool(name="ps_s", bufs=4, space="PSUM") as ps_s,
                tc.tile_pool(name="ps_acc", bufs=2, space="PSUM") as ps_acc,
                tc.tile_pool(name="ps_sum", bufs=2, space="PSUM") as ps_sum,
            ):
                NAHEAD = 3
                head_tiles = {0: (qt_h0, kt_h0, v_h0)}

                def get_head(h):
                    if h not in head_tiles:
                        qt_h = hpool.tile([128, L], F32R, tag="qt_h")
                        nc.sync.dma_start(qt_h[:], qt_parts[h][:])
                        kt_h = hpool.tile([128, L], F32R, tag="kt_h")
                        nc.sync.dma_start(kt_h[:], kt_parts[h][:])
                        v_h = hpool.tile([128, 16, 128], F32R, tag="v_h")
                        nc.sync.dma_start(v_h[:], v_parts[h - 1][:])
                        head_tiles[h] = (qt_h, kt_h, v_h)
                    return head_tiles[h]

                NSTEP = NHL * 4 * 16  # 256
                p_tiles = {}

                def step_of(g):
                    return g // 64, (g // 16) % 4, g % 16  # h, qb, kt

                def emit_scores(g):
                    h, qb, kt = step_of(g)
                    qt_h, kt_h, _ = get_head(h)
                    sp = ps_s.tile([128, 512], F32, tag="sp")
                    nc.tensor.matmul(
                        sp[:],
                        lhsT=kt_h[:, kt * 128 : (kt + 1) * 128],
                        rhs=qt_h[:, qb * 512 : (qb + 1) * 512],
                        start=True,
                        stop=True,
                    )
                    p_sb = ppool.tile([128, 512], F32R, tag="p")
                    nc.scalar.activation(p_sb[:], sp[:], AF.Exp, scale=SCALE)
                    p_tiles[g] = p_sb

                pending_tail = None

                def flush_tail():
                    nonlocal pending_tail
                    if pending_tail is None:
                        return
                    av, recip, h, qb = pending_tail
                    rb_sb = spool.tile([128, 512], F32R, tag="rb")
                    nc.gpsimd.partition_broadcast(rb_sb[:], recip[:])
                    nc.vector.tensor_mul(ot_qb[qb][:, h, :], av[:], rb_sb[:])
                    pending_tail = None

                for g in range(NAHEAD):
                    emit_scores(g)
                av = acc = ssum = None
                for g in range(NSTEP):
                    h, qb, kt = step_of(g)
                    if kt == 0:
                        if qb == 0:
                            for hn in (h + 1, h + 2):
                                if hn < NHL:
                                    get_head(hn)  # prefetch ahead
                        av = ps_acc.tile([128, 512], F32, tag="av")
                        acc = apool.tile([128, 512], F32R, tag="acc")
                        ssum = ps_sum.tile([1, 512], F32, tag="ssum")
                    p_sb = p_tiles.pop(g)
                    _, _, v_h = head_tiles[h]
                    nc.tensor.matmul(
                        av[:],
                        lhsT=v_h[:, kt, :],
                        rhs=p_sb[:],
                        start=(kt == 0),
                        stop=(kt == 15),
                    )
                    if kt % 2 == 0:
                        nc.tensor.matmul(
                            ssum[:],
                            lhsT=ones[:, 0:1],
                            rhs=p_sb[:],
                            start=(kt == 0),
                            stop=(kt == 15),
                        )
                    else:
                        with nc.allow_low_precision(reason="f32r strip rowsum"):
                            if kt == 1:
                                nc.vector.tensor_copy(acc[:], p_sb[:])
                            else:
                                nc.vector.tensor_add(acc[:], acc[:], p_sb[:])
                    if g + NAHEAD < NSTEP:
                        emit_scores(g + NAHEAD)
                    if kt == 2:
                        flush_tail()
                    if kt == 15:
                        # fold the DVE half-chain into the PE partial sums
                        nc.tensor.matmul(
                            ssum[:],
                            lhsT=ones[:, 0:1],
                            rhs=acc[:],
                            start=False,
                            stop=True,
                        )
                        recip = spool.tile([1, 512], F32R, tag="recip")
                        with nc.allow_low_precision(reason="f32r feeds matmul"):
                            nc.vector.reciprocal(recip[:], ssum[:])
                        pending_tail = (av, recip, h, qb)
                        if qb == 3:
                            head_tiles.pop(h)
                flush_tail()

            # ---------- Phase 3: O projection (partial sums) ----------
            with (
                tc.tile_pool(name="ev3", bufs=4) as ev3,
                tc.tile_pool(name="ps3", bufs=4, space="PSUM") as ps3,
            ):
                for qt in range(16):
                    qb3, qt_local = qt // 4, qt % 4
                    for dst, wsb in ((pr, wor_sb), (pi, woi_sb)):
                        for nb in range(2):
                            ps = ps3.tile([128, 512], F32, tag="ps3")
                            for h in range(NHL):
                                nc.tensor.matmul(
                                    ps[:],
                                    lhsT=ot_qb[qb3][
                                        :, h, qt_local * 128 : (qt_local + 1) * 128
                                    ],
                                    rhs=wsb[:, h, nb * 512 : (nb + 1) * 512],
                                    start=(h == 0),
                                    stop=(h == NHL - 1),
                                )
                            ev = ev3.tile([128, 512], F32, tag="ev3")
                            nc.scalar.activation(ev[:], ps[:], AF.Copy)
                            nc.sync.dma_start(
                                dst[qt * 128 : (qt + 1) * 128, nb * 512 : (nb + 1) * 512],
                                ev[:],
                            )
            wo_ctx.__exit__(None, None, None)
            h0_ctx.__exit__(None, None, None)
    if not nc.is_finalized():
        nc.finalize()
    return nc


_NC = None


def _get_nc():
    global _NC
    if _NC is None:
        _NC = _build_nc()
    return _NC


def _prep(inputs):
    f = lambda k: np.asarray(inputs[k], np.float32)
    zr, zi = f("zr"), f("zi")
    w = {n: f(n) for n in inputs if n not in ("zr", "zi")}

    z3t = [
        np.ascontiguousarray(
            np.concatenate([zr[b].T, zi[b].T, (zr[b] + zi[b]).T], axis=0)
        )
        for b in range(B)
    ]

    in_maps = []
    for c in range(N_CORES):
        b, hg = c // 4, c % 4
        Ch = np.arange(hg * 4 * HD, (hg * 4 + 4) * HD)  # 256 local channels
        m = {"z3t": z3t[b]}
        for name in ("q", "k", "v"):
            wr, wi = w[f"w{name}_r"], w[f"w{name}_i"]
            A = wr[Ch, :].T  # [1024, 256]
            Bm = wi[Ch, :].T
            wg = np.empty((3 * D, 256), np.float32)
            wg[0:D] = Bm - A  # m=0: zr chain  (Gauss m2)
            wg[D : 2 * D] = Bm + A  # m=1: zi chain  (Gauss m3)
            wg[2 * D :] = A  # m=2: zs chain  (Gauss m1)
            m[f"w{name}g"] = wg
            if name != "v":
                br, bi = w[f"b{name}_r"], w[f"b{name}_i"]
                m[f"c{name}r"] = (br[Ch] - bi[Ch]).astype(np.float32)
                m[f"c{name}i"] = (br[Ch] + bi[Ch]).astype(np.float32)
        wo_r, wo_i = w["wo_r"], w["wo_i"]
        wor = np.empty((CH2, D), np.float32)
        woi = np.empty((CH2, D), np.float32)
        for l in range(NHL):
            Chl = np.arange((hg * 4 + l) * HD, (hg * 4 + l + 1) * HD)
            s = l * 128
            wor[s : s + 64, :] = wo_r[:, Chl].T
            wor[s + 64 : s + 128, :] = -wo_i[:, Chl].T
            woi[s : s + 64, :] = wo_i[:, Chl].T
            woi[s + 64 : s + 128, :] = wo_r[:, Chl].T
        m["wor"] = wor
        m["woi"] = woi
        in_maps.append(m)

    # exact host-side bias: V-bias folds through softmax (rows sum to 1)
    cvr = w["bv_r"] - w["bv_i"]
    cvi = w["bv_r"] + w["bv_i"]
    br_total = w["wo_r"] @ cvr - w["wo_i"] @ cvi + w["bo_r"] - w["bo_i"]
    bi_total = w["wo_r"] @ cvi + w["wo_i"] @ cvr + w["bo_r"] + w["bo_i"]
    return in_maps, br_total.astype(np.float32), bi_total.astype(np.float32)


LAST_RESULTS = None


def kernel(**inputs):
    global LAST_RESULTS
    nc = _get_nc()
    in_maps, br_total, bi_total = _prep(inputs)
    res = run_bass_kernel_spmd(nc, in_maps, core_ids=list(range(N_CORES)))
    LAST_RESULTS = res
    out_r = np.zeros((B, L, D), np.float32)
    out_i = np.zeros((B, L, D), np.float32)
    for c in range(N_CORES):
        out_r[c // 4] += res.results[c]["pr"]
        out_i[c // 4] += res.results[c]["pi"]
    out_r += br_total[None, None, :]
    out_i += bi_total[None, None, :]
    return out_r, out_i



# revision 21
# speedup vs baseline: 1.0058x; 1.0058x over previous
"""ComplexMultiheadAttention on 8 Trainium2 NeuronCores.

Sharding: core c handles batch b = c//4 and the 4 heads [4*(c%4), 4*(c%4)+4).
QKV use Gauss's 3-multiplication complex trick: with A = wr[Ch].T and
B = wi[Ch].T, the three K=1024 chains  m2 = zr(B-A), m3 = zi(B+A), m1 = zsA
(zs = zr+zi precomputed on host) give real = m1-m3, imag = m1+m2 — 25% less
PE work than the fused K=2048 form.  The DVE folds bias + combine + PSUM
evacuation into one scalar_tensor_tensor op per 64-channel half.
The O-projection is row-parallel (Megatron): each core emits a partial
[2048,1024] sum; the host adds the 4 partials per batch plus the exact
bias term (V-bias folds into the output bias because softmax rows sum to 1).
"""

import os
import sys

import numpy as np

sys.path.insert(0, "/opt/trn_rl_repo")

import concourse.bass as bass
import concourse.bacc as bacc_mod
import concourse.mybir as mybir
from concourse.bass_utils import run_bass_kernel_spmd
from concourse.tile import TileContext

try:  # tracing needs antenv.axon_hooks (test harness injects it)
    import antenv.axon_hooks  # noqa: F401
except ImportError:
    os.environ.setdefault("BASS_NEVER_TRACE", "1")

B, L, D, NH = 2, 2048, 1024, 16
HD = D // NH  # 64
N_CORES = 8
NHL = 4  # heads per core
CH2 = 2 * NHL * HD  # 512 local channels, per-head [r(64), i(64)] interleaved
F2 = 2 * D  # 2048 concat feature dim
SCALE = 1.0 / 8.0  # 1/sqrt(HD)

F32 = mybir.dt.float32
F32R = mybir.dt.float32r
AF = mybir.ActivationFunctionType


def _build_nc():
    nc = bacc_mod.Bacc(None, target_bir_lowering=False, debug=False)
    # z3t rows: m=0: zr^T, m=1: zi^T, m=2: (zr+zi)^T
    z3t = nc.declare_dram_parameter("z3t", [3 * D, L], F32R, isOutput=False)
    # w?g rows (m*1024+f): m=0: (wi-wr)[Ch].T, m=1: (wi+wr)[Ch].T, m=2: wr[Ch].T
    wqg = nc.declare_dram_parameter("wqg", [3 * D, 256], F32R, isOutput=False)
    wkg = nc.declare_dram_parameter("wkg", [3 * D, 256], F32R, isOutput=False)
    wv = nc.declare_dram_parameter("wv", [F2, CH2], F32R, isOutput=False)
    cqr = nc.declare_dram_parameter("cqr", [256], F32, isOutput=False)
    cqi = nc.declare_dram_parameter("cqi", [256], F32, isOutput=False)
    ckr = nc.declare_dram_parameter("ckr", [256], F32, isOutput=False)
    cki = nc.declare_dram_parameter("cki", [256], F32, isOutput=False)
    wor = nc.declare_dram_parameter("wor", [CH2, D], F32R, isOutput=False)
    woi = nc.declare_dram_parameter("woi", [CH2, D], F32R, isOutput=False)
    pr = nc.declare_dram_parameter("pr", [L, D], F32, isOutput=True)
    pi = nc.declare_dram_parameter("pi", [L, D], F32, isOutput=True)
    ALU = mybir.AluOpType

    NSB = 4
    SBW = L // NSB  # 512 seq cols per block

    with TileContext(nc) as tc:
        with (
            tc.tile_pool(name="dram", bufs=1, space="DRAM") as dpool,
            tc.tile_pool(name="persist", bufs=1) as pers,
        ):
            # per-head contiguous spill tiles -> single big-descriptor loads
            qt_parts = [
                dpool.tile([128, L], F32R, tag=f"qtp_{h}", name=f"qtp_{h}")
                for h in range(NHL)
            ]
            kt_parts = [
                dpool.tile([128, L], F32R, tag=f"ktp_{h}", name=f"ktp_{h}")
                for h in range(NHL)
            ]
            v_parts = [
                dpool.tile([128, 16, 128], F32R, tag=f"vp_{h}", name=f"vp_{h}")
                for h in range(1, NHL)
            ]

            ones_f = pers.tile([128, 1], F32, tag="ones_f")
            nc.vector.memset(ones_f[:], 1.0)
            ones = pers.tile([128, 1], F32R, tag="ones")
            nc.scalar.activation(ones[:], ones_f[:], AF.Copy)
            onesr_f = pers.tile([1, 128], F32, tag="onesr_f")
            nc.vector.memset(onesr_f[:], 1.0)
            onesr = pers.tile([1, 128], F32R, tag="onesr")
            nc.scalar.activation(onesr[:], onesr_f[:], AF.Copy)
            warm = pers.tile([128, 8], F32R, tag="warm")
            bias_sb = {}
            for nm, dram in (("cqr", cqr), ("cqi", cqi), ("ckr", ckr), ("cki", cki)):
                t = pers.tile([128, 2], F32, tag=nm)
                nc.sync.dma_start(t[:], dram[:].rearrange("(t p) -> p t", p=128))
                bias_sb[nm] = t

            # ---------- Phase 1: QKV projections ----------
            # QT/KT in [ch, seq] layout; V in [seq, ch] layout.
            # Head 0's Q/K land directly in SBUF (no DRAM round-trip) so
            # phase 2 starts with zero DMA latency.
            h0_ctx = tc.tile_pool(name="head0", bufs=1)
            h0pool = h0_ctx.__enter__()
            qt_h0 = h0pool.tile([128, L], F32R, tag="qt_h0")
            kt_h0 = h0pool.tile([128, L], F32R, tag="kt_h0")
            v_h0 = h0pool.tile([128, 16, 128], F32R, tag="v_h0")
            with (
                tc.tile_pool(name="w1", bufs=1) as wpool,
                tc.tile_pool(name="z3", bufs=40) as zpool,
                tc.tile_pool(name="ev1", bufs=6) as ev1,
                tc.tile_pool(name="m1sp", bufs=3) as m1sp,
                tc.tile_pool(name="evv", bufs=2) as evv,
                tc.tile_pool(name="ps_qk", bufs=5, space="PSUM") as ps_qk,
                tc.tile_pool(name="ps_v", bufs=3, space="PSUM") as ps_v,
            ):
                wq_sb = wpool.tile([128, 24, 256], F32R, tag="wq")
                wk_sb = wpool.tile([128, 24, 256], F32R, tag="wk")
                wv_sb = wpool.tile([128, 16, CH2], F32R, tag="wv")

                zcs = {}  # (sb, m, f8) -> [128, SBW] quarter tile

                def z3_q(sb, m, f8):
                    t = zpool.tile([128, SBW], F32R, tag="zc")
                    nc.sync.dma_start(
                        t[:],
                        z3t[
                            m * D + f8 * 128 : m * D + (f8 + 1) * 128,
                            sb * SBW : (sb + 1) * SBW,
                        ],
                    )
                    zcs[(sb, m, f8)] = t

                def wslice(wsb, wd, i):
                    nc.sync.dma_start(wsb[:, i, :], wd[i * 128 : (i + 1) * 128, :])

                # critical-prefix DMA order: interleave the first chain's z
                # quarters with its weight slices so the PE starts after
                # ~400 KB and streams at consumption order thereafter.
                for f8 in range(8):
                    z3_q(0, 0, f8)
                    wslice(wq_sb, wqg, f8)
                for f8 in range(8):
                    z3_q(0, 1, f8)
                    wslice(wq_sb, wqg, 8 + f8)
                for f8 in range(8):
                    z3_q(0, 2, f8)
                    wslice(wq_sb, wqg, 16 + f8)
                for i in range(24):
                    wslice(wk_sb, wkg, i)
                for i in range(16):
                    wslice(wv_sb, wv, i)

                for sb in range(NSB):
                    sc = slice(sb * SBW, (sb + 1) * SBW)
                    for wsb, cr, ci, parts, direct in (
                        (wq_sb, bias_sb["cqr"], bias_sb["cqi"], qt_parts, qt_h0),
                        (wk_sb, bias_sb["ckr"], bias_sb["cki"], kt_parts, kt_h0),
                    ):
                        for ct in range(2):
                            ps = [None] * 3
                            for m in range(3):
                                ps[m] = ps_qk.tile(
                                    [128, SBW], F32, tag="psqk", name=f"psqk_{m}"
                                )
                                for f8 in range(8):
                                    nc.tensor.matmul(
                                        ps[m][:],
                                        lhsT=wsb[
                                            :, m * 8 + f8, ct * 128 : (ct + 1) * 128
                                        ],
                                        rhs=zcs[(sb, m, f8)][:],
                                        start=(f8 == 0),
                                        stop=(f8 == 7),
                                    )
                            # per head: real = m1-m3+cbr, imag = m1+m2+cbi
                            # (ps[2]=m1 zs-chain, ps[1]=m3 zi, ps[0]=m2 zr);
                            # m1 goes PSUM->SBUF first (TensorScalarPtr may
                            # read only one PSUM operand)
                            m1s = m1sp.tile([128, SBW], F32R, tag="m1s", name="m1s")
                            nc.scalar.activation(m1s[:], ps[2][:], AF.Copy)
                            for hh in range(2):
                                h = 2 * ct + hh
                                pp = slice(hh * 64, (hh + 1) * 64)
                                if h == 0:
                                    dr = direct[0:64, sc]
                                    di = direct[64:128, sc]
                                else:
                                    stg = ev1.tile([128, SBW], F32R, tag="ev1")
                                    dr = stg[0:64, :]
                                    di = stg[64:128, :]
                                with nc.allow_low_precision(reason="f32r qkv"):
                                    nc.vector.scalar_tensor_tensor(
                                        dr,
                                        m1s[pp, :],
                                        cr[pp, ct : ct + 1],
                                        ps[1][pp, :],
                                        op0=ALU.add,
                                        op1=ALU.subtract,
                                    )
                                    nc.vector.scalar_tensor_tensor(
                                        di,
                                        m1s[pp, :],
                                        ci[pp, ct : ct + 1],
                                        ps[0][pp, :],
                                        op0=ALU.add,
                                        op1=ALU.add,
                                    )
                                if h > 0:
                                    nc.sync.dma_start(parts[h][:, sc], stg[:])
                    for st in range(SBW // 128):
                        if sb < NSB - 1:
                            # prefetch next block's z quarters (6 per st)
                            for j in range(6):
                                mi, fi = divmod(st * 6 + j, 8)
                                z3_q(sb + 1, mi, fi)
                        ps = ps_v.tile([128, CH2], F32, tag="psv", name="psv")
                        for ft in range(16):
                            nc.tensor.matmul(
                                ps[:],
                                lhsT=zcs[(sb, ft // 8, ft % 8)][
                                    :, st * 128 : (st + 1) * 128
                                ],
                                rhs=wv_sb[:, ft, :],
                                start=(ft == 0),
                                stop=(ft == 15),
                            )
                        ev = evv.tile([128, CH2], F32R, tag="vstage")
                        nc.scalar.activation(ev[:], ps[:], AF.Copy)
                        ktidx = sb * 4 + st
                        nc.sync.dma_start(v_h0[:, ktidx, :], ev[:, 0:128])
                        for hh in range(1, NHL):
                            nc.sync.dma_start(
                                v_parts[hh - 1][:, ktidx, :],
                                ev[:, hh * 128 : (hh + 1) * 128],
                            )

            # ---------- Phase 2: attention, flat software pipeline ----------
            # Rowsums ride the (otherwise idle) DVE as a running elementwise
            # sum of the 16 exp strips; one ones-matmul per (h,qb) block
            # reduces the accumulated strip across partitions.
            # warm up the GpSimd custom-instruction library before phase 2
            # (first partition_broadcast otherwise pays a ~10us IRAM load)
            nc.gpsimd.partition_broadcast(warm[:], onesr[0:1, 0:8])
            wo_ctx = tc.tile_pool(name="wo", bufs=1)
            wopool = wo_ctx.__enter__()
            wor_sb = wopool.tile([128, NHL, D], F32R, tag="wor")
            woi_sb = wopool.tile([128, NHL, D], F32R, tag="woi")
            for hh in range(NHL):
                nc.sync.dma_start(wor_sb[:, hh, :], wor[hh * 128 : (hh + 1) * 128, :])
                nc.sync.dma_start(woi_sb[:, hh, :], woi[hh * 128 : (hh + 1) * 128, :])
            # per-qb OT tiles: [128 ch, head, 512 q], per-head [or(64), oi(64)]
            ot_qb = [
                wopool.tile([128, NHL, 512], F32R, tag=f"ot_{qb}", name=f"ot_{qb}")
                for qb in range(4)
            ]

            with (
                tc.tile_pool(name="heads", bufs=3) as hpool,
                tc.tile_pool(name="pstrips", bufs=6) as ppool,
                tc.tile_pool(name="accs", bufs=4) as apool,
                tc.tile_pool(name="small2", bufs=4) as spool,
                tc.tile_pool(name="ps_s", bufs=4, space="PSUM") as ps_s,
                tc.tile_pool(name="ps_acc", bufs=2, space="PSUM") as ps_acc,
                tc.tile_pool(name="ps_sum", bufs=2, space="PSUM") as ps_sum,
            ):
                NAHEAD = 3
                head_tiles = {0: (qt_h0, kt_h0, v_h0)}

                def get_head(h):
                    if h not in head_tiles:
                        qt_h = hpool.tile([128, L], F32R, tag="qt_h")
                        nc.sync.dma_start(qt_h[:], qt_parts[h][:])
                        kt_h = hpool.tile([128, L], F32R, tag="kt_h")
                        nc.sync.dma_start(kt_h[:], kt_parts[h][:])
                        v_h = hpool.tile([128, 16, 128], F32R, tag="v_h")
                        nc.sync.dma_start(v_h[:], v_parts[h - 1][:])
                        head_tiles[h] = (qt_h, kt_h, v_h)
                    return head_tiles[h]

                NSTEP = NHL * 4 * 16  # 256
                p_tiles = {}

                def step_of(g):
                    return g // 64, (g // 16) % 4, g % 16  # h, qb, kt

                def emit_scores(g):
                    h, qb, kt = step_of(g)
                    qt_h, kt_h, _ = get_head(h)
                    sp = ps_s.tile([128, 512], F32, tag="sp")
                    nc.tensor.matmul(
                        sp[:],
                        lhsT=kt_h[:, kt * 128 : (kt + 1) * 128],
                        rhs=qt_h[:, qb * 512 : (qb + 1) * 512],
                        start=True,
                        stop=True,
                    )
                    p_sb = ppool.tile([128, 512], F32R, tag="p")
                    nc.scalar.activation(p_sb[:], sp[:], AF.Exp, scale=SCALE)
                    p_tiles[g] = p_sb

                pending_tail = None

                def flush_tail():
                    nonlocal pending_tail
                    if pending_tail is None:
                        return
                    av, recip, h, qb = pending_tail
                    rb_sb = spool.tile([128, 512], F32R, tag="rb")
                    nc.gpsimd.partition_broadcast(rb_sb[:], recip[:])
                    nc.vector.tensor_mul(ot_qb[qb][:, h, :], av[:], rb_sb[:])
                    pending_tail = None

                for g in range(NAHEAD):
                    emit_scores(g)
                av = acc = ssum = None
                for g in range(NSTEP):
                    h, qb, kt = step_of(g)
                    if kt == 0:
                        if qb == 0:
                            for hn in (h + 1, h + 2):
                                if hn < NHL:
                                    get_head(hn)  # prefetch ahead
                        av = ps_acc.tile([128, 512], F32, tag="av")
                        acc = apool.tile([128, 512], F32R, tag="acc")
                        ssum = ps_sum.tile([1, 512], F32, tag="ssum")
                    p_sb = p_tiles.pop(g)
                    _, _, v_h = head_tiles[h]
                    nc.tensor.matmul(
                        av[:],
                        lhsT=v_h[:, kt, :],
                        rhs=p_sb[:],
                        start=(kt == 0),
                        stop=(kt == 15),
                    )
                    if kt % 2 == 0:
                        nc.tensor.matmul(
                            ssum[:],
                            lhsT=ones[:, 0:1],
                            rhs=p_sb[:],
                            start=(kt == 0),
                            stop=(kt == 15),
                        )
                    else:
                        with nc.allow_low_precision(reason="f32r strip rowsum"):
                            if kt == 1:
                                nc.vector.tensor_copy(acc[:], p_sb[:])
                            else:
                                nc.vector.tensor_add(acc[:], acc[:], p_sb[:])
                    if g + NAHEAD < NSTEP:
                        emit_scores(g + NAHEAD)
                    if kt == 2:
                        flush_tail()
                    if kt == 15:
                        # fold the DVE half-chain into the PE partial sums
                        nc.tensor.matmul(
                            ssum[:],
                            lhsT=ones[:, 0:1],
                            rhs=acc[:],
                            start=False,
                            stop=True,
                        )
                        recip = spool.tile([1, 512], F32R, tag="recip")
                        with nc.allow_low_precision(reason="f32r feeds matmul"):
                            nc.vector.reciprocal(recip[:], ssum[:])
                        pending_tail = (av, recip, h, qb)
                        if qb == 3:
                            head_tiles.pop(h)
                flush_tail()

            # ---------- Phase 3: O projection (partial sums) ----------
            with (
                tc.tile_pool(name="ev3", bufs=4) as ev3,
                tc.tile_pool(name="ps3", bufs=4, space="PSUM") as ps3,
            ):
                for qt in range(16):
                    qb3, qt_local = qt // 4, qt % 4
                    for dst, wsb in ((pr, wor_sb), (pi, woi_sb)):
                        for nb in range(2):
                            ps = ps3.tile([128, 512], F32, tag="ps3")
                            for h in range(NHL):
                                nc.tensor.matmul(
                                    ps[:],
                                    lhsT=ot_qb[qb3][
                                        :, h, qt_local * 128 : (qt_local + 1) * 128
                                    ],
                                    rhs=wsb[:, h, nb * 512 : (nb + 1) * 512],
                                    start=(h == 0),
                                    stop=(h == NHL - 1),
                                )
                            ev = ev3.tile([128, 512], F32, tag="ev3")
                            nc.scalar.activation(ev[:], ps[:], AF.Copy)
                            nc.sync.dma_start(
                                dst[qt * 128 : (qt + 1) * 128, nb * 512 : (nb + 1) * 512],
                                ev[:],
                            )
            wo_ctx.__exit__(None, None, None)
            h0_ctx.__exit__(None, None, None)
    if not nc.is_finalized():
        nc.finalize()
    return nc


_NC = None


def _get_nc():
    global _NC
    if _NC is None:
        _NC = _build_nc()
    return _NC


def _prep(inputs):
    f = lambda k: np.asarray(inputs[k], np.float32)
    zr, zi = f("zr"), f("zi")
    w = {n: f(n) for n in inputs if n not in ("zr", "zi")}

    z3t = [
        np.ascontiguousarray(
            np.concatenate([zr[b].T, zi[b].T, (zr[b] + zi[b]).T], axis=0)
        )
        for b in range(B)
    ]

    in_maps = []
    for c in range(N_CORES):
        b, hg = c // 4, c % 4
        Ch = np.arange(hg * 4 * HD, (hg * 4 + 4) * HD)  # 256 local channels
        m = {"z3t": z3t[b]}
        for name in ("q", "k"):
            wr, wi = w[f"w{name}_r"], w[f"w{name}_i"]
            A = wr[Ch, :].T  # [1024, 256]
            Bm = wi[Ch, :].T
            wg = np.empty((3 * D, 256), np.float32)
            wg[0:D] = Bm - A  # m=0: zr chain  (Gauss m2)
            wg[D : 2 * D] = Bm + A  # m=1: zi chain  (Gauss m3)
            wg[2 * D :] = A  # m=2: zs chain  (Gauss m1)
            m[f"w{name}g"] = wg
            br, bi = w[f"b{name}_r"], w[f"b{name}_i"]
            m[f"c{name}r"] = (br[Ch] - bi[Ch]).astype(np.float32)
            m[f"c{name}i"] = (br[Ch] + bi[Ch]).astype(np.float32)
        wr, wi = w["wv_r"], w["wv_i"]
        wcat = np.empty((F2, CH2), np.float32)
        for l in range(NHL):
            Chl = np.arange((hg * 4 + l) * HD, (hg * 4 + l + 1) * HD)
            s = l * 128
            wcat[:D, s : s + 64] = wr[Chl, :].T
            wcat[D:, s : s + 64] = -wi[Chl, :].T
            wcat[:D, s + 64 : s + 128] = wi[Chl, :].T
            wcat[D:, s + 64 : s + 128] = wr[Chl, :].T
        m["wv"] = wcat
        wo_r, wo_i = w["wo_r"], w["wo_i"]
        wor = np.empty((CH2, D), np.float32)
        woi = np.empty((CH2, D), np.float32)
        for l in range(NHL):
            Chl = np.arange((hg * 4 + l) * HD, (hg * 4 + l + 1) * HD)
            s = l * 128
            wor[s : s + 64, :] = wo_r[:, Chl].T
            wor[s + 64 : s + 128, :] = -wo_i[:, Chl].T
            woi[s : s + 64, :] = wo_i[:, Chl].T
            woi[s + 64 : s + 128, :] = wo_r[:, Chl].T
        m["wor"] = wor
        m["woi"] = woi
        in_maps.append(m)

    # exact host-side bias: V-bias folds through softmax (rows sum to 1)
    cvr = w["bv_r"] - w["bv_i"]
    cvi = w["bv_r"] + w["bv_i"]
    br_total = w["wo_r"] @ cvr - w["wo_i"] @ cvi + w["bo_r"] - w["bo_i"]
    bi_total = w["wo_r"] @ cvi + w["wo_i"] @ cvr + w["bo_r"] + w["bo_i"]
    return in_maps, br_total.astype(np.float32), bi_total.astype(np.float32)


LAST_RESULTS = None


def kernel(**inputs):
    global LAST_RESULTS
    nc = _get_nc()
    in_maps, br_total, bi_total = _prep(inputs)
    res = run_bass_kernel_spmd(nc, in_maps, core_ids=list(range(N_CORES)))
    LAST_RESULTS = res
    out_r = np.zeros((B, L, D), np.float32)
    out_i = np.zeros((B, L, D), np.float32)
    for c in range(N_CORES):
        out_r[c // 4] += res.results[c]["pr"]
        out_i[c // 4] += res.results[c]["pi"]
    out_r += br_total[None, None, :]
    out_i += bi_total[None, None, :]
    return out_r, out_i



# revision 22
# speedup vs baseline: 1.0179x; 1.0120x over previous
"""ComplexMultiheadAttention on 8 Trainium2 NeuronCores.

Sharding: core c handles batch b = c//4 and the 4 heads [4*(c%4), 4*(c%4)+4).
QKV use Gauss's 3-multiplication complex trick: with A = wr[Ch].T and
B = wi[Ch].T, the three K=1024 chains  m2 = zr(B-A), m3 = zi(B+A), m1 = zsA
(zs = zr+zi precomputed on host) give real = m1-m3, imag = m1+m2 — 25% less
PE work than the fused K=2048 form.  The DVE folds bias + combine + PSUM
evacuation into one scalar_tensor_tensor op per 64-channel half.
The O-projection is row-parallel (Megatron): each core emits a partial
[2048,1024] sum; the host adds the 4 partials per batch plus the exact
bias term (V-bias folds into the output bias because softmax rows sum to 1).
"""

import os
import sys

import numpy as np

sys.path.insert(0, "/opt/trn_rl_repo")

import concourse.bass as bass
import concourse.bacc as bacc_mod
import concourse.mybir as mybir
from concourse.bass_utils import run_bass_kernel_spmd
from concourse.tile import TileContext

try:  # tracing needs antenv.axon_hooks (test harness injects it)
    import antenv.axon_hooks  # noqa: F401
except ImportError:
    os.environ.setdefault("BASS_NEVER_TRACE", "1")

B, L, D, NH = 2, 2048, 1024, 16
HD = D // NH  # 64
N_CORES = 8
NHL = 4  # heads per core
CH2 = 2 * NHL * HD  # 512 local channels, per-head [r(64), i(64)] interleaved
F2 = 2 * D  # 2048 concat feature dim
SCALE = 1.0 / 8.0  # 1/sqrt(HD)

F32 = mybir.dt.float32
F32R = mybir.dt.float32r
AF = mybir.ActivationFunctionType


def _build_nc():
    nc = bacc_mod.Bacc(None, target_bir_lowering=False, debug=False)
    # z3t rows: m=0: zr^T, m=1: zi^T, m=2: (zr+zi)^T
    z3t = nc.declare_dram_parameter("z3t", [3 * D, L], F32R, isOutput=False)
    # w?g rows (m*1024+f): m=0: (wi-wr)[Ch].T, m=1: (wi+wr)[Ch].T, m=2: wr[Ch].T
    wqg = nc.declare_dram_parameter("wqg", [3 * D, 256], F32R, isOutput=False)
    wkg = nc.declare_dram_parameter("wkg", [3 * D, 256], F32R, isOutput=False)
    wv = nc.declare_dram_parameter("wv", [F2, CH2], F32R, isOutput=False)
    cqr = nc.declare_dram_parameter("cqr", [256], F32, isOutput=False)
    cqi = nc.declare_dram_parameter("cqi", [256], F32, isOutput=False)
    ckr = nc.declare_dram_parameter("ckr", [256], F32, isOutput=False)
    cki = nc.declare_dram_parameter("cki", [256], F32, isOutput=False)
    wor = nc.declare_dram_parameter("wor", [CH2, D], F32R, isOutput=False)
    woi = nc.declare_dram_parameter("woi", [CH2, D], F32R, isOutput=False)
    pr = nc.declare_dram_parameter("pr", [L, D], F32, isOutput=True)
    pi = nc.declare_dram_parameter("pi", [L, D], F32, isOutput=True)
    ALU = mybir.AluOpType

    NSB = 4
    SBW = L // NSB  # 512 seq cols per block

    with TileContext(nc) as tc:
        with (
            tc.tile_pool(name="dram", bufs=1, space="DRAM") as dpool,
            tc.tile_pool(name="persist", bufs=1) as pers,
        ):
            # per-head contiguous spill tiles -> single big-descriptor loads
            qt_parts = [
                dpool.tile([128, L], F32R, tag=f"qtp_{h}", name=f"qtp_{h}")
                for h in range(NHL)
            ]
            kt_parts = [
                dpool.tile([128, L], F32R, tag=f"ktp_{h}", name=f"ktp_{h}")
                for h in range(NHL)
            ]
            v_parts = [
                dpool.tile([128, 16, 128], F32R, tag=f"vp_{h}", name=f"vp_{h}")
                for h in range(1, NHL)
            ]

            ones_f = pers.tile([128, 1], F32, tag="ones_f")
            nc.vector.memset(ones_f[:], 1.0)
            ones = pers.tile([128, 1], F32R, tag="ones")
            nc.scalar.activation(ones[:], ones_f[:], AF.Copy)
            onesr_f = pers.tile([1, 128], F32, tag="onesr_f")
            nc.vector.memset(onesr_f[:], 1.0)
            onesr = pers.tile([1, 128], F32R, tag="onesr")
            nc.scalar.activation(onesr[:], onesr_f[:], AF.Copy)
            warm = pers.tile([128, 8], F32R, tag="warm")
            bias_sb = {}
            for nm, dram in (("cqr", cqr), ("cqi", cqi), ("ckr", ckr), ("cki", cki)):
                t = pers.tile([128, 2], F32, tag=nm)
                nc.sync.dma_start(t[:], dram[:].rearrange("(t p) -> p t", p=128))
                bias_sb[nm] = t

            # ---------- Phase 1: QKV projections ----------
            # QT/KT in [ch, seq] layout; V in [seq, ch] layout.
            # Head 0's Q/K land directly in SBUF (no DRAM round-trip) so
            # phase 2 starts with zero DMA latency.
            h0_ctx = tc.tile_pool(name="head0", bufs=1)
            h0pool = h0_ctx.__enter__()
            qt_h0 = h0pool.tile([128, L], F32R, tag="qt_h0")
            kt_h0 = h0pool.tile([128, L], F32R, tag="kt_h0")
            v_h0 = h0pool.tile([128, 16, 128], F32R, tag="v_h0")
            with (
                tc.tile_pool(name="w1", bufs=1) as wpool,
                tc.tile_pool(name="z3", bufs=40) as zpool,
                tc.tile_pool(name="ev1", bufs=6) as ev1,
                tc.tile_pool(name="m1sp", bufs=3) as m1sp,
                tc.tile_pool(name="evv", bufs=2) as evv,
                tc.tile_pool(name="ps_qk", bufs=6, space="PSUM") as ps_qk,
                tc.tile_pool(name="ps_v", bufs=2, space="PSUM") as ps_v,
            ):
                wq_sb = wpool.tile([128, 24, 256], F32R, tag="wq")
                wk_sb = wpool.tile([128, 24, 256], F32R, tag="wk")
                wv_sb = wpool.tile([128, 16, CH2], F32R, tag="wv")

                zcs = {}  # (sb, m, f8) -> [128, SBW] quarter tile

                def z3_q(sb, m, f8):
                    t = zpool.tile([128, SBW], F32R, tag="zc")
                    nc.sync.dma_start(
                        t[:],
                        z3t[
                            m * D + f8 * 128 : m * D + (f8 + 1) * 128,
                            sb * SBW : (sb + 1) * SBW,
                        ],
                    )
                    zcs[(sb, m, f8)] = t

                def wslice(wsb, wd, i):
                    nc.sync.dma_start(wsb[:, i, :], wd[i * 128 : (i + 1) * 128, :])

                # critical-prefix DMA order: interleave the first chain's z
                # quarters with its weight slices so the PE starts after
                # ~400 KB and streams at consumption order thereafter.
                for f8 in range(8):
                    z3_q(0, 0, f8)
                    wslice(wq_sb, wqg, f8)
                for f8 in range(8):
                    z3_q(0, 1, f8)
                    wslice(wq_sb, wqg, 8 + f8)
                for f8 in range(8):
                    z3_q(0, 2, f8)
                    wslice(wq_sb, wqg, 16 + f8)
                for i in range(24):
                    wslice(wk_sb, wkg, i)
                for i in range(16):
                    wslice(wv_sb, wv, i)

                for sb in range(NSB):
                    sc = slice(sb * SBW, (sb + 1) * SBW)
                    for wsb, cr, ci, parts, direct in (
                        (wq_sb, bias_sb["cqr"], bias_sb["cqi"], qt_parts, qt_h0),
                        (wk_sb, bias_sb["ckr"], bias_sb["cki"], kt_parts, kt_h0),
                    ):
                        for ct in range(2):
                            ps = [None] * 3
                            for m in range(3):
                                ps[m] = ps_qk.tile(
                                    [128, SBW], F32, tag="psqk", name=f"psqk_{m}"
                                )
                                for f8 in range(8):
                                    nc.tensor.matmul(
                                        ps[m][:],
                                        lhsT=wsb[
                                            :, m * 8 + f8, ct * 128 : (ct + 1) * 128
                                        ],
                                        rhs=zcs[(sb, m, f8)][:],
                                        start=(f8 == 0),
                                        stop=(f8 == 7),
                                    )
                            # per head: real = m1-m3+cbr, imag = m1+m2+cbi
                            # (ps[2]=m1 zs-chain, ps[1]=m3 zi, ps[0]=m2 zr);
                            # m1 goes PSUM->SBUF first (TensorScalarPtr may
                            # read only one PSUM operand)
                            m1s = m1sp.tile([128, SBW], F32R, tag="m1s", name="m1s")
                            nc.scalar.activation(m1s[:], ps[2][:], AF.Copy)
                            for hh in range(2):
                                h = 2 * ct + hh
                                pp = slice(hh * 64, (hh + 1) * 64)
                                if h == 0:
                                    dr = direct[0:64, sc]
                                    di = direct[64:128, sc]
                                else:
                                    stg = ev1.tile([128, SBW], F32R, tag="ev1")
                                    dr = stg[0:64, :]
                                    di = stg[64:128, :]
                                with nc.allow_low_precision(reason="f32r qkv"):
                                    nc.vector.scalar_tensor_tensor(
                                        dr,
                                        m1s[pp, :],
                                        cr[pp, ct : ct + 1],
                                        ps[1][pp, :],
                                        op0=ALU.add,
                                        op1=ALU.subtract,
                                    )
                                    nc.vector.scalar_tensor_tensor(
                                        di,
                                        m1s[pp, :],
                                        ci[pp, ct : ct + 1],
                                        ps[0][pp, :],
                                        op0=ALU.add,
                                        op1=ALU.add,
                                    )
                                if h > 0:
                                    nc.sync.dma_start(parts[h][:, sc], stg[:])
                    for st in range(SBW // 128):
                        if sb < NSB - 1:
                            # prefetch next block's z quarters (6 per st)
                            for j in range(6):
                                mi, fi = divmod(st * 6 + j, 8)
                                z3_q(sb + 1, mi, fi)
                        ps = ps_v.tile([128, CH2], F32, tag="psv", name="psv")
                        for ft in range(16):
                            nc.tensor.matmul(
                                ps[:],
                                lhsT=zcs[(sb, ft // 8, ft % 8)][
                                    :, st * 128 : (st + 1) * 128
                                ],
                                rhs=wv_sb[:, ft, :],
                                start=(ft == 0),
                                stop=(ft == 15),
                            )
                        ev = evv.tile([128, CH2], F32R, tag="vstage")
                        nc.scalar.activation(ev[:], ps[:], AF.Copy)
                        ktidx = sb * 4 + st
                        nc.sync.dma_start(v_h0[:, ktidx, :], ev[:, 0:128])
                        for hh in range(1, NHL):
                            nc.sync.dma_start(
                                v_parts[hh - 1][:, ktidx, :],
                                ev[:, hh * 128 : (hh + 1) * 128],
                            )

            # ---------- Phase 2: attention, flat software pipeline ----------
            # Rowsums ride the (otherwise idle) DVE as a running elementwise
            # sum of the 16 exp strips; one ones-matmul per (h,qb) block
            # reduces the accumulated strip across partitions.
            # warm up the GpSimd custom-instruction library before phase 2
            # (first partition_broadcast otherwise pays a ~10us IRAM load)
            nc.gpsimd.partition_broadcast(warm[:], onesr[0:1, 0:8])
            wo_ctx = tc.tile_pool(name="wo", bufs=1)
            wopool = wo_ctx.__enter__()
            wor_sb = wopool.tile([128, NHL, D], F32R, tag="wor")
            woi_sb = wopool.tile([128, NHL, D], F32R, tag="woi")
            for hh in range(NHL):
                nc.sync.dma_start(wor_sb[:, hh, :], wor[hh * 128 : (hh + 1) * 128, :])
                nc.sync.dma_start(woi_sb[:, hh, :], woi[hh * 128 : (hh + 1) * 128, :])
            # per-qb OT tiles: [128 ch, head, 512 q], per-head [or(64), oi(64)]
            ot_qb = [
                wopool.tile([128, NHL, 512], F32R, tag=f"ot_{qb}", name=f"ot_{qb}")
                for qb in range(4)
            ]

            with (
                tc.tile_pool(name="heads", bufs=3) as hpool,
                tc.tile_pool(name="pstrips", bufs=6) as ppool,
                tc.tile_pool(name="accs", bufs=4) as apool,
                tc.tile_pool(name="small2", bufs=4) as spool,
                tc.tile_pool(name="ps_s", bufs=4, space="PSUM") as ps_s,
                tc.tile_pool(name="ps_acc", bufs=2, space="PSUM") as ps_acc,
                tc.tile_pool(name="ps_sum", bufs=2, space="PSUM") as ps_sum,
            ):
                NAHEAD = 3
                head_tiles = {0: (qt_h0, kt_h0, v_h0)}

                def get_head(h):
                    if h not in head_tiles:
                        qt_h = hpool.tile([128, L], F32R, tag="qt_h")
                        nc.sync.dma_start(qt_h[:], qt_parts[h][:])
                        kt_h = hpool.tile([128, L], F32R, tag="kt_h")
                        nc.sync.dma_start(kt_h[:], kt_parts[h][:])
                        v_h = hpool.tile([128, 16, 128], F32R, tag="v_h")
                        nc.sync.dma_start(v_h[:], v_parts[h - 1][:])
                        head_tiles[h] = (qt_h, kt_h, v_h)
                    return head_tiles[h]

                NSTEP = NHL * 4 * 16  # 256
                p_tiles = {}

                def step_of(g):
                    return g // 64, (g // 16) % 4, g % 16  # h, qb, kt

                def emit_scores(g):
                    h, qb, kt = step_of(g)
                    qt_h, kt_h, _ = get_head(h)
                    sp = ps_s.tile([128, 512], F32, tag="sp")
                    nc.tensor.matmul(
                        sp[:],
                        lhsT=kt_h[:, kt * 128 : (kt + 1) * 128],
                        rhs=qt_h[:, qb * 512 : (qb + 1) * 512],
                        start=True,
                        stop=True,
                    )
                    p_sb = ppool.tile([128, 512], F32R, tag="p")
                    nc.scalar.activation(p_sb[:], sp[:], AF.Exp, scale=SCALE)
                    p_tiles[g] = p_sb

                pending_tail = None

                def flush_tail():
                    nonlocal pending_tail
                    if pending_tail is None:
                        return
                    av, recip, h, qb = pending_tail
                    rb_sb = spool.tile([128, 512], F32R, tag="rb")
                    nc.gpsimd.partition_broadcast(rb_sb[:], recip[:])
                    nc.vector.tensor_mul(ot_qb[qb][:, h, :], av[:], rb_sb[:])
                    pending_tail = None

                for g in range(NAHEAD):
                    emit_scores(g)
                av = acc = ssum = None
                for g in range(NSTEP):
                    h, qb, kt = step_of(g)
                    if kt == 0:
                        if qb == 0:
                            for hn in (h + 1, h + 2):
                                if hn < NHL:
                                    get_head(hn)  # prefetch ahead
                        av = ps_acc.tile([128, 512], F32, tag="av")
                        acc = apool.tile([128, 512], F32R, tag="acc")
                        ssum = ps_sum.tile([1, 512], F32, tag="ssum")
                    p_sb = p_tiles.pop(g)
                    _, _, v_h = head_tiles[h]
                    nc.tensor.matmul(
                        av[:],
                        lhsT=v_h[:, kt, :],
                        rhs=p_sb[:],
                        start=(kt == 0),
                        stop=(kt == 15),
                    )
                    if kt % 2 == 0:
                        nc.tensor.matmul(
                            ssum[:],
                            lhsT=ones[:, 0:1],
                            rhs=p_sb[:],
                            start=(kt == 0),
                            stop=(kt == 15),
                        )
                    else:
                        with nc.allow_low_precision(reason="f32r strip rowsum"):
                            if kt == 1:
                                nc.vector.tensor_copy(acc[:], p_sb[:])
                            else:
                                nc.vector.tensor_add(acc[:], acc[:], p_sb[:])
                    if g + NAHEAD < NSTEP:
                        emit_scores(g + NAHEAD)
                    if kt == 2:
                        flush_tail()
                    if kt == 15:
                        # fold the DVE half-chain into the PE partial sums
                        nc.tensor.matmul(
                            ssum[:],
                            lhsT=ones[:, 0:1],
                            rhs=acc[:],
                            start=False,
                            stop=True,
                        )
                        recip = spool.tile([1, 512], F32R, tag="recip")
                        with nc.allow_low_precision(reason="f32r feeds matmul"):
                            nc.vector.reciprocal(recip[:], ssum[:])
                        pending_tail = (av, recip, h, qb)
                        if qb == 3:
                            head_tiles.pop(h)
                flush_tail()

            # ---------- Phase 3: O projection (partial sums) ----------
            with (
                tc.tile_pool(name="ev3", bufs=4) as ev3,
                tc.tile_pool(name="ps3", bufs=4, space="PSUM") as ps3,
            ):
                for qt in range(16):
                    qb3, qt_local = qt // 4, qt % 4
                    for dst, wsb in ((pr, wor_sb), (pi, woi_sb)):
                        for nb in range(2):
                            ps = ps3.tile([128, 512], F32, tag="ps3")
                            for h in range(NHL):
                                nc.tensor.matmul(
                                    ps[:],
                                    lhsT=ot_qb[qb3][
                                        :, h, qt_local * 128 : (qt_local + 1) * 128
                                    ],
                                    rhs=wsb[:, h, nb * 512 : (nb + 1) * 512],
                                    start=(h == 0),
                                    stop=(h == NHL - 1),
                                )
                            ev = ev3.tile([128, 512], F32, tag="ev3")
                            nc.scalar.activation(ev[:], ps[:], AF.Copy)
                            nc.sync.dma_start(
                                dst[qt * 128 : (qt + 1) * 128, nb * 512 : (nb + 1) * 512],
                                ev[:],
                            )
            wo_ctx.__exit__(None, None, None)
            h0_ctx.__exit__(None, None, None)
    if not nc.is_finalized():
        nc.finalize()
    return nc


_NC = None


def _get_nc():
    global _NC
    if _NC is None:
        _NC = _build_nc()
    return _NC


def _prep(inputs):
    f = lambda k: np.asarray(inputs[k], np.float32)
    zr, zi = f("zr"), f("zi")
    w = {n: f(n) for n in inputs if n not in ("zr", "zi")}

    z3t = [
        np.ascontiguousarray(
            np.concatenate([zr[b].T, zi[b].T, (zr[b] + zi[b]).T], axis=0)
        )
        for b in range(B)
    ]

    in_maps = []
    for c in range(N_CORES):
        b, hg = c // 4, c % 4
        Ch = np.arange(hg * 4 * HD, (hg * 4 + 4) * HD)  # 256 local channels
        m = {"z3t": z3t[b]}
        for name in ("q", "k"):
            wr, wi = w[f"w{name}_r"], w[f"w{name}_i"]
            A = wr[Ch, :].T  # [1024, 256]
            Bm = wi[Ch, :].T
            wg = np.empty((3 * D, 256), np.float32)
            wg[0:D] = Bm - A  # m=0: zr chain  (Gauss m2)
            wg[D : 2 * D] = Bm + A  # m=1: zi chain  (Gauss m3)
            wg[2 * D :] = A  # m=2: zs chain  (Gauss m1)
            m[f"w{name}g"] = wg
            br, bi = w[f"b{name}_r"], w[f"b{name}_i"]
            m[f"c{name}r"] = (br[Ch] - bi[Ch]).astype(np.float32)
            m[f"c{name}i"] = (br[Ch] + bi[Ch]).astype(np.float32)
        wr, wi = w["wv_r"], w["wv_i"]
        wcat = np.empty((F2, CH2), np.float32)
        for l in range(NHL):
            Chl = np.arange((hg * 4 + l) * HD, (hg * 4 + l + 1) * HD)
            s = l * 128
            wcat[:D, s : s + 64] = wr[Chl, :].T
            wcat[D:, s : s + 64] = -wi[Chl, :].T
            wcat[:D, s + 64 : s + 128] = wi[Chl, :].T
            wcat[D:, s + 64 : s + 128] = wr[Chl, :].T
        m["wv"] = wcat
        wo_r, wo_i = w["wo_r"], w["wo_i"]
        wor = np.empty((CH2, D), np.float32)
        woi = np.empty((CH2, D), np.float32)
        for l in range(NHL):
            Chl = np.arange((hg * 4 + l) * HD, (hg * 4 + l + 1) * HD)
            s = l * 128
            wor[s : s + 64, :] = wo_r[:, Chl].T
            wor[s + 64 : s + 128, :] = -wo_i[:, Chl].T
            woi[s : s + 64, :] = wo_i[:, Chl].T
            woi[s + 64 : s + 128, :] = wo_r[:, Chl].T
        m["wor"] = wor
        m["woi"] = woi
        in_maps.append(m)

    # exact host-side bias: V-bias folds through softmax (rows sum to 1)
    cvr = w["bv_r"] - w["bv_i"]
    cvi = w["bv_r"] + w["bv_i"]
    br_total = w["wo_r"] @ cvr - w["wo_i"] @ cvi + w["bo_r"] - w["bo_i"]
    bi_total = w["wo_r"] @ cvi + w["wo_i"] @ cvr + w["bo_r"] + w["bo_i"]
    return in_maps, br_total.astype(np.float32), bi_total.astype(np.float32)


LAST_RESULTS = None


def kernel(**inputs):
    global LAST_RESULTS
    nc = _get_nc()
    in_maps, br_total, bi_total = _prep(inputs)
    res = run_bass_kernel_spmd(nc, in_maps, core_ids=list(range(N_CORES)))
    LAST_RESULTS = res
    out_r = np.zeros((B, L, D), np.float32)
    out_i = np.zeros((B, L, D), np.float32)
    for c in range(N_CORES):
        out_r[c // 4] += res.results[c]["pr"]
        out_i[c // 4] += res.results[c]["pi"]
    out_r += br_total[None, None, :]
    out_i += bi_total[None, None, :]
    return out_r, out_i

